# revision 7
# baseline (speedup 1.0000x reference)
"""Trainium2 kernel for nn_DifferentiableRenderer: batch-parallel point
projection + z-buffer scatter (last-write-wins).

Sharding: pure data parallel — B=16 images across 8 NeuronCores (2 each).
Device computes the memory-bound projection (world->camera transform,
perspective divide, pixel index + validity) for all 500K points per image;
per-pixel winner resolution is applied on the gathered per-point
(pixel, depth) arrays.
"""

import numpy as np

# ---------------------------------------------------------------------------
# TileContext compatibility patch: the walrus build in this environment
# rejects instructions carrying more than one sync-wait ("Too many sync wait
# commands") and Drain instructions with waits. Replace the Tile kernel-tail
# drain+barrier, and split any multi-wait instruction that slips through.
# ---------------------------------------------------------------------------


def _install_tile_patch():
    from concourse.tile import TileContext
    from concourse.vector_clock import ScopedClock, VectorClock

    if getattr(TileContext, "_render_patch", False):
        return

    def _patched_drain_and_barrier(self, tick_clock, wait_clock):
        nc = self.nc
        vec = list(tick_clock.global_clock)
        for proc, tick in enumerate(vec):
            if tick > 0:
                v = [0] * len(vec)
                v[proc] = tick
                nop = nc.sync.nop(nofuse=True)
                wait_clock.add_sem_waits(
                    nop.ins, ScopedClock({None: VectorClock(v)})
                )
        nc.all_engine_barrier(sem_only=True)
        popped = nc._tile_sem_poison_stack.pop()
        assert popped is self._sem_poison
        sems = list(self.sems.allocated().values())
        sem_nums = sorted(s.num if hasattr(s, "num") else int(s) for s in sems)
        if sem_nums:
            from concourse.bass import compact_to_ranges

            for r in compact_to_ranges(sem_nums):
                nc.gpsimd.sem_clear(r)
            nc._state.prepend_free_semaphores(sem_nums)
            for poison_set in nc._tile_sem_poison_stack:
                poison_set.update(sem_nums)
        nc.all_engine_barrier(sem_only=True)

    _orig_lower = TileContext._lower_ordered_insts

    def _split_multi_waits(self, ordered):
        import concourse.mybir as mybir

        for bb_name, insts in ordered.items():
            i = 0
            while i < len(insts):
                ins = insts[i]
                si = ins.sync_info
                if si is not None and len(si.on_wait) > 1:
                    waits = list(si.on_wait)
                    carriers = []
                    for w in waits[:-1]:
                        nop = mybir.InstNoOp(
                            name=f"I-{self.nc.next_id()}-ws", ins=[], outs=[]
                        )
                        nop.engine = ins.engine
                        nop.sync_info = mybir.SyncInfo(on_wait=[w], on_update=[])
                        carriers.append(nop)
                    ins.sync_info = mybir.SyncInfo(
                        on_wait=[waits[-1]], on_update=list(si.on_update)
                    )
                    insts[i:i] = carriers
                    i += len(carriers)
                i += 1
        return ordered

    def _patched_lower(self, ordered):
        return _orig_lower(self, _split_multi_waits(self, ordered))

    TileContext._drain_and_barrier = _patched_drain_and_barrier
    TileContext._lower_ordered_insts = _patched_lower
    TileContext._render_patch = True


# ---------------------------------------------------------------------------
# Problem constants (hardcoded per the task contract)
# ---------------------------------------------------------------------------
B, N = 16, 500000
H, W = 224, 224
N_CORES = 8
IMGS_PER_CORE = B // N_CORES  # 2
NPAD = ((N + 127) // 128) * 128  # 500096, multiple of 128
COLS = NPAD // 128  # 3907 columns per partition per image
TILE = 512
NTILES = (COLS + TILE - 1) // TILE

_NC_CACHE = {}
LAST_RESULTS = None


def _build_nc():
    """Per-core Bass program: for each of 2 images, project NPAD points ->
    per-point pixel index (int32, OOB=H*W) and depth (f32)."""
    import concourse.bass as bass
    import concourse.mybir as mybir
    from concourse.tile import TileContext

    _install_tile_patch()

    nc = bass.Bass()
    f32 = mybir.dt.float32
    Alu = mybir.AluOpType
    verts = nc.dram_tensor(
        "verts", [IMGS_PER_CORE, 128, COLS * 3], f32, kind="ExternalInput"
    )
    # 16 scalars per image, pre-replicated across 128 partitions on host
    consts = nc.dram_tensor(
        "consts", [IMGS_PER_CORE, 128, 16], f32, kind="ExternalInput"
    )
    pix_out = nc.dram_tensor(
        "pix", [IMGS_PER_CORE, 128, COLS], mybir.dt.int32, kind="ExternalOutput"
    )
    dep_out = nc.dram_tensor(
        "dep", [IMGS_PER_CORE, 128, COLS], f32, kind="ExternalOutput"
    )

    with TileContext(nc) as tc:
        with (
            tc.tile_pool(name="io", bufs=3) as io_pool,
            tc.tile_pool(name="wk", bufs=2) as wk_pool,
            tc.tile_pool(name="cs", bufs=1) as cs_pool,
        ):
            cvec = []
            for img in range(IMGS_PER_CORE):
                cbc = cs_pool.tile([128, 16], f32, tag=f"cbc{img}")
                nc.sync.dma_start(out=cbc[:], in_=consts[img])
                cvec.append(cbc)

            for img in range(IMGS_PER_CORE):
                cb = cvec[img]
                r00, r01, r02 = cb[:, 0:1], cb[:, 1:2], cb[:, 2:3]
                r10, r11, r12 = cb[:, 3:4], cb[:, 4:5], cb[:, 5:6]
                r20, r21, r22 = cb[:, 6:7], cb[:, 7:8], cb[:, 8:9]
                tx, ty, tz = cb[:, 9:10], cb[:, 10:11], cb[:, 11:12]
                fx, fy = cb[:, 12:13], cb[:, 13:14]
                cx, cy = cb[:, 14:15], cb[:, 15:16]

                for t in range(NTILES):
                    lo = t * TILE
                    hi = min(COLS, lo + TILE)
                    F = hi - lo
                    raw = io_pool.tile([128, TILE * 3], f32, tag="raw")
                    nc.sync.dma_start(
                        out=raw[:, : F * 3],
                        in_=verts[img, :, lo * 3 : hi * 3],
                    )
                    rawv = raw[:, : F * 3].rearrange("p (f c) -> p f c", c=3)
                    x = wk_pool.tile([128, TILE], f32, tag="x")
                    y = wk_pool.tile([128, TILE], f32, tag="y")
                    z = wk_pool.tile([128, TILE], f32, tag="z")
                    nc.vector.tensor_copy(out=x[:, :F], in_=rawv[:, :, 0])
                    nc.vector.tensor_copy(out=y[:, :F], in_=rawv[:, :, 1])
                    nc.scalar.copy(out=z[:, :F], in_=rawv[:, :, 2])

                    xs, ys, zs = x[:, :F], y[:, :F], z[:, :F]

                    vcx = wk_pool.tile([128, TILE], f32, tag="vcx")
                    vcy = wk_pool.tile([128, TILE], f32, tag="vcy")
                    vcz = wk_pool.tile([128, TILE], f32, tag="vcz")
                    tmp = wk_pool.tile([128, TILE], f32, tag="tmp")

                    def mad3(out, ra, rb, rc, tt):
                        # out = ((x*ra + y*rb) + z*rc) + tt -- same order as
                        # the reference einsum + translation add
                        nc.vector.tensor_scalar_mul(out[:, :F], xs, ra)
                        nc.vector.scalar_tensor_tensor(
                            out[:, :F], ys, rb, out[:, :F], Alu.mult, Alu.add
                        )
                        nc.vector.scalar_tensor_tensor(
                            out[:, :F], zs, rc, out[:, :F], Alu.mult, Alu.add
                        )
                        nc.vector.tensor_scalar(
                            out[:, :F], out[:, :F], tt, None, Alu.add
                        )

                    mad3(vcx, r00, r01, r02, tx)
                    mad3(vcy, r10, r11, r12, ty)
                    mad3(vcz, r20, r21, r22, tz)

                    # zr = 1/(vcz + 1e-8) on ACT, one DVE Newton refinement
                    zb = wk_pool.tile([128, TILE], f32, tag="zb")
                    nc.vector.tensor_scalar(
                        zb[:, :F], vcz[:, :F], 1e-8, None, Alu.add
                    )
                    zr = wk_pool.tile([128, TILE], f32, tag="zr")
                    nc.vector.reciprocal(out=zr[:, :F], in_=zb[:, :F])
                    nc.vector.tensor_mul(tmp[:, :F], zb[:, :F], zr[:, :F])
                    nc.vector.tensor_scalar(
                        tmp[:, :F], tmp[:, :F], 2.0, -1.0,
                        Alu.subtract, Alu.mult,
                    )
                    nc.vector.tensor_mul(zr[:, :F], zr[:, :F], tmp[:, :F])

                    u = wk_pool.tile([128, TILE], f32, tag="u")
                    v = wk_pool.tile([128, TILE], f32, tag="v")
                    nc.vector.tensor_mul(u[:, :F], vcx[:, :F], zr[:, :F])
                    nc.vector.tensor_scalar(
                        u[:, :F], u[:, :F], fx, cx, Alu.mult, Alu.add
                    )
                    nc.vector.tensor_mul(v[:, :F], vcy[:, :F], zr[:, :F])
                    nc.vector.tensor_scalar(
                        v[:, :F], v[:, :F], fy, cy, Alu.mult, Alu.add
                    )

                    # valid = (u > -1) & (u < W) & (v > -1) & (v < H);
                    # NaN compares false -> invalid
                    msk = wk_pool.tile([128, TILE], f32, tag="msk")
                    nc.vector.tensor_scalar(
                        msk[:, :F], u[:, :F], -1.0, None, Alu.is_gt
                    )
                    nc.vector.scalar_tensor_tensor(
                        msk[:, :F], u[:, :F], float(W), msk[:, :F],
                        Alu.is_lt, Alu.logical_and,
                    )
                    nc.vector.scalar_tensor_tensor(
                        msk[:, :F], v[:, :F], -1.0, msk[:, :F],
                        Alu.is_gt, Alu.logical_and,
                    )
                    nc.vector.scalar_tensor_tensor(
                        msk[:, :F], v[:, :F], float(H), msk[:, :F],
                        Alu.is_lt, Alu.logical_and,
                    )

                    # floor of clamped coords (== trunc for the valid range):
                    # uc = clamp(u,0,hi); r = roundcast(uc); r -= (r > uc)
                    ui = wk_pool.tile([128, TILE], f32, tag="ui")
                    vi = wk_pool.tile([128, TILE], f32, tag="vi")
                    iu = wk_pool.tile([128, TILE], mybir.dt.int32, tag="iu")
                    iv = wk_pool.tile([128, TILE], mybir.dt.int32, tag="iv")
                    rf = wk_pool.tile([128, TILE], f32, tag="rf")
                    rg = wk_pool.tile([128, TILE], f32, tag="rg")

                    def floor_clamped(dst, src, hi_val, itile, rtile):
                        nc.vector.tensor_scalar(
                            dst[:, :F], src[:, :F], 0.0, hi_val,
                            Alu.max, Alu.min,
                        )
                        nc.scalar.copy(out=itile[:, :F], in_=dst[:, :F])
                        nc.scalar.copy(out=rtile[:, :F], in_=itile[:, :F])
                        nc.vector.tensor_tensor(
                            out=itile[:, :F].bitcast(f32),
                            in0=rtile[:, :F],
                            in1=dst[:, :F],
                            op=Alu.is_gt,
                        )
                        nc.vector.tensor_sub(
                            dst[:, :F], rtile[:, :F], itile[:, :F].bitcast(f32)
                        )

                    floor_clamped(ui, u, float(W), iu, rf)
                    floor_clamped(vi, v, float(H), iv, rg)

                    # pix = valid ? vi*W + ui : H*W   (+H*W fused into cast)
                    pixf = wk_pool.tile([128, TILE], f32, tag="pixf")
                    nc.vector.scalar_tensor_tensor(
                        pixf[:, :F], vi[:, :F], float(W), ui[:, :F],
                        Alu.mult, Alu.add,
                    )
                    nc.vector.tensor_scalar(
                        pixf[:, :F], pixf[:, :F], float(H * W), None,
                        Alu.subtract,
                    )
                    nc.vector.tensor_mul(pixf[:, :F], pixf[:, :F], msk[:, :F])
                    pixi = wk_pool.tile([128, TILE], mybir.dt.int32, tag="pixi")
                    nc.scalar.activation(
                        out=pixi[:, :F],
                        in_=pixf[:, :F],
                        func=mybir.ActivationFunctionType.Copy,
                        bias=float(H * W),
                    )

                    nc.sync.dma_start(
                        out=pix_out[img, :, lo:hi], in_=pixi[:, :F]
                    )
                    nc.sync.dma_start(
                        out=dep_out[img, :, lo:hi], in_=vcz[:, :F]
                    )
    return nc


def _get_nc():
    if "nc" not in _NC_CACHE:
        _NC_CACHE["nc"] = _build_nc()
    return _NC_CACHE["nc"]


def kernel(vertices, rotation, translation, camera_intrinsics):
    global LAST_RESULTS
    from concourse.bass_utils import run_bass_kernel_spmd

    vertices = np.ascontiguousarray(vertices, dtype=np.float32)
    rotation = np.asarray(rotation, dtype=np.float32)
    translation = np.asarray(translation, dtype=np.float32)
    camera_intrinsics = np.asarray(camera_intrinsics, dtype=np.float32)

    in_maps = []
    for core in range(N_CORES):
        vimgs = []
        cimgs = []
        for j in range(IMGS_PER_CORE):
            b = core * IMGS_PER_CORE + j
            vp = np.full((NPAD, 3), np.nan, dtype=np.float32)
            vp[:N] = vertices[b]
            # device layout: partition p holds points [p*COLS, (p+1)*COLS)
            vdev = vp.reshape(128, COLS * 3)
            vimgs.append(vdev)
            R = rotation[b]
            K = camera_intrinsics[b]
            c = np.zeros(16, dtype=np.float32)
            c[0:9] = R.reshape(9)
            c[9:12] = translation[b]
            c[12], c[13] = K[0, 0], K[1, 1]
            c[14], c[15] = K[0, 2], K[1, 2]
            cimgs.append(np.broadcast_to(c, (128, 16)).copy())
        in_maps.append(
            {"verts": np.stack(vimgs), "consts": np.stack(cimgs)}
        )

    nc = _get_nc()
    import time as _time

    _t0 = _time.time()
    res = run_bass_kernel_spmd(nc, in_maps, core_ids=list(range(N_CORES)))
    globals()["LAST_EXEC_S"] = _time.time() - _t0
    LAST_RESULTS = res

    out = np.zeros((B, 1, H, W), dtype=np.float32)
    flat = out.reshape(B, H * W)
    for core in range(N_CORES):
        r = res.results[core]
        for j in range(IMGS_PER_CORE):
            b = core * IMGS_PER_CORE + j
            pixv = r["pix"][j].reshape(128 * COLS)[:N]
            depv = r["dep"][j].reshape(128 * COLS)[:N]
            m = (pixv >= 0) & (pixv < H * W)
            # sequential fancy assignment: later duplicates overwrite earlier
            flat[b][pixv[m]] = depv[m]
    return out


# revision 8
# speedup vs baseline: 1.0442x; 1.0442x over previous
"""Trainium2 kernel for nn_DifferentiableRenderer: batch-parallel point
projection + z-buffer scatter (last-write-wins).

Sharding: pure data parallel — B=16 images across 8 NeuronCores (2 each).
Device computes the memory-bound projection (world->camera transform,
perspective divide, pixel index + validity) for all 500K points per image;
per-pixel winner resolution is applied on the gathered per-point
(pixel, depth) arrays.
"""

import numpy as np

# ---------------------------------------------------------------------------
# TileContext compatibility patch: the walrus build in this environment
# rejects instructions carrying more than one sync-wait ("Too many sync wait
# commands") and Drain instructions with waits. Replace the Tile kernel-tail
# drain+barrier, and split any multi-wait instruction that slips through.
# ---------------------------------------------------------------------------


def _install_tile_patch():
    from concourse.tile import TileContext
    from concourse.vector_clock import ScopedClock, VectorClock

    if getattr(TileContext, "_render_patch", False):
        return

    def _patched_drain_and_barrier(self, tick_clock, wait_clock):
        nc = self.nc
        vec = list(tick_clock.global_clock)
        for proc, tick in enumerate(vec):
            if tick > 0:
                v = [0] * len(vec)
                v[proc] = tick
                nop = nc.sync.nop(nofuse=True)
                wait_clock.add_sem_waits(
                    nop.ins, ScopedClock({None: VectorClock(v)})
                )
        nc.all_engine_barrier(sem_only=True)
        popped = nc._tile_sem_poison_stack.pop()
        assert popped is self._sem_poison
        sems = list(self.sems.allocated().values())
        sem_nums = sorted(s.num if hasattr(s, "num") else int(s) for s in sems)
        if sem_nums:
            from concourse.bass import compact_to_ranges

            for r in compact_to_ranges(sem_nums):
                nc.gpsimd.sem_clear(r)
            nc._state.prepend_free_semaphores(sem_nums)
            for poison_set in nc._tile_sem_poison_stack:
                poison_set.update(sem_nums)
        nc.all_engine_barrier(sem_only=True)

    _orig_lower = TileContext._lower_ordered_insts

    def _split_multi_waits(self, ordered):
        import concourse.mybir as mybir

        for bb_name, insts in ordered.items():
            i = 0
            while i < len(insts):
                ins = insts[i]
                si = ins.sync_info
                if si is not None and len(si.on_wait) > 1:
                    waits = list(si.on_wait)
                    carriers = []
                    for w in waits[:-1]:
                        nop = mybir.InstNoOp(
                            name=f"I-{self.nc.next_id()}-ws", ins=[], outs=[]
                        )
                        nop.engine = ins.engine
                        nop.sync_info = mybir.SyncInfo(on_wait=[w], on_update=[])
                        carriers.append(nop)
                    ins.sync_info = mybir.SyncInfo(
                        on_wait=[waits[-1]], on_update=list(si.on_update)
                    )
                    insts[i:i] = carriers
                    i += len(carriers)
                i += 1
        return ordered

    def _patched_lower(self, ordered):
        return _orig_lower(self, _split_multi_waits(self, ordered))

    TileContext._drain_and_barrier = _patched_drain_and_barrier
    TileContext._lower_ordered_insts = _patched_lower
    TileContext._render_patch = True


# ---------------------------------------------------------------------------
# Problem constants (hardcoded per the task contract)
# ---------------------------------------------------------------------------
B, N = 16, 500000
H, W = 224, 224
N_CORES = 8
IMGS_PER_CORE = B // N_CORES  # 2
NPAD = ((N + 127) // 128) * 128  # 500096, multiple of 128
COLS = NPAD // 128  # 3907 columns per partition per image
TILE = 512
NTILES = (COLS + TILE - 1) // TILE

_NC_CACHE = {}
LAST_RESULTS = None


def _build_nc():
    """Per-core Bass program: for each of 2 images, project NPAD points ->
    per-point pixel index (int32, OOB=H*W) and depth (f32)."""
    import concourse.bass as bass
    import concourse.mybir as mybir
    from concourse.tile import TileContext

    _install_tile_patch()

    nc = bass.Bass()
    f32 = mybir.dt.float32
    Alu = mybir.AluOpType
    vx_in = nc.dram_tensor(
        "vx", [IMGS_PER_CORE, 128, COLS], f32, kind="ExternalInput"
    )
    vy_in = nc.dram_tensor(
        "vy", [IMGS_PER_CORE, 128, COLS], f32, kind="ExternalInput"
    )
    vz_in = nc.dram_tensor(
        "vz", [IMGS_PER_CORE, 128, COLS], f32, kind="ExternalInput"
    )
    # 16 scalars per image, pre-replicated across 128 partitions on host
    consts = nc.dram_tensor(
        "consts", [IMGS_PER_CORE, 128, 16], f32, kind="ExternalInput"
    )
    pix_out = nc.dram_tensor(
        "pix", [IMGS_PER_CORE, 128, COLS], mybir.dt.int32, kind="ExternalOutput"
    )
    dep_out = nc.dram_tensor(
        "dep", [IMGS_PER_CORE, 128, COLS], f32, kind="ExternalOutput"
    )

    with TileContext(nc) as tc:
        with (
            tc.tile_pool(name="io", bufs=3) as io_pool,
            tc.tile_pool(name="wk", bufs=2) as wk_pool,
            tc.tile_pool(name="cs", bufs=1) as cs_pool,
        ):
            cvec = []
            for img in range(IMGS_PER_CORE):
                cbc = cs_pool.tile([128, 16], f32, tag=f"cbc{img}")
                nc.sync.dma_start(out=cbc[:], in_=consts[img])
                cvec.append(cbc)

            for img in range(IMGS_PER_CORE):
                cb = cvec[img]
                r00, r01, r02 = cb[:, 0:1], cb[:, 1:2], cb[:, 2:3]
                r10, r11, r12 = cb[:, 3:4], cb[:, 4:5], cb[:, 5:6]
                r20, r21, r22 = cb[:, 6:7], cb[:, 7:8], cb[:, 8:9]
                tx, ty, tz = cb[:, 9:10], cb[:, 10:11], cb[:, 11:12]
                fx, fy = cb[:, 12:13], cb[:, 13:14]
                cx, cy = cb[:, 14:15], cb[:, 15:16]

                for t in range(NTILES):
                    lo = t * TILE
                    hi = min(COLS, lo + TILE)
                    F = hi - lo
                    x = io_pool.tile([128, TILE], f32, tag="x")
                    y = io_pool.tile([128, TILE], f32, tag="y")
                    z = io_pool.tile([128, TILE], f32, tag="z")
                    nc.sync.dma_start(out=x[:, :F], in_=vx_in[img, :, lo:hi])
                    nc.sync.dma_start(out=y[:, :F], in_=vy_in[img, :, lo:hi])
                    nc.sync.dma_start(out=z[:, :F], in_=vz_in[img, :, lo:hi])

                    xs, ys, zs = x[:, :F], y[:, :F], z[:, :F]

                    vcx = wk_pool.tile([128, TILE], f32, tag="vcx")
                    vcy = wk_pool.tile([128, TILE], f32, tag="vcy")
                    vcz = wk_pool.tile([128, TILE], f32, tag="vcz")
                    tmp = wk_pool.tile([128, TILE], f32, tag="tmp")

                    def mad3(out, ra, rb, rc, tt):
                        # out = ((x*ra + y*rb) + z*rc) + tt -- same order as
                        # the reference einsum + translation add
                        nc.vector.tensor_scalar_mul(out[:, :F], xs, ra)
                        nc.vector.scalar_tensor_tensor(
                            out[:, :F], ys, rb, out[:, :F], Alu.mult, Alu.add
                        )
                        nc.vector.scalar_tensor_tensor(
                            out[:, :F], zs, rc, out[:, :F], Alu.mult, Alu.add
                        )
                        nc.vector.tensor_scalar(
                            out[:, :F], out[:, :F], tt, None, Alu.add
                        )

                    mad3(vcx, r00, r01, r02, tx)
                    mad3(vcy, r10, r11, r12, ty)
                    mad3(vcz, r20, r21, r22, tz)

                    # zr = 1/(vcz + 1e-8) on ACT, one DVE Newton refinement
                    zb = wk_pool.tile([128, TILE], f32, tag="zb")
                    nc.vector.tensor_scalar(
                        zb[:, :F], vcz[:, :F], 1e-8, None, Alu.add
                    )
                    zr = wk_pool.tile([128, TILE], f32, tag="zr")
                    nc.vector.reciprocal(out=zr[:, :F], in_=zb[:, :F])
                    nc.vector.tensor_mul(tmp[:, :F], zb[:, :F], zr[:, :F])
                    nc.vector.tensor_scalar(
                        tmp[:, :F], tmp[:, :F], 2.0, -1.0,
                        Alu.subtract, Alu.mult,
                    )
                    nc.vector.tensor_mul(zr[:, :F], zr[:, :F], tmp[:, :F])

                    u = wk_pool.tile([128, TILE], f32, tag="u")
                    v = wk_pool.tile([128, TILE], f32, tag="v")
                    nc.vector.tensor_mul(u[:, :F], vcx[:, :F], zr[:, :F])
                    nc.vector.tensor_scalar(
                        u[:, :F], u[:, :F], fx, cx, Alu.mult, Alu.add
                    )
                    nc.vector.tensor_mul(v[:, :F], vcy[:, :F], zr[:, :F])
                    nc.vector.tensor_scalar(
                        v[:, :F], v[:, :F], fy, cy, Alu.mult, Alu.add
                    )

                    # valid = (u > -1) & (u < W) & (v > -1) & (v < H);
                    # NaN compares false -> invalid
                    msk = wk_pool.tile([128, TILE], f32, tag="msk")
                    nc.vector.tensor_scalar(
                        msk[:, :F], u[:, :F], -1.0, None, Alu.is_gt
                    )
                    nc.vector.scalar_tensor_tensor(
                        msk[:, :F], u[:, :F], float(W), msk[:, :F],
                        Alu.is_lt, Alu.logical_and,
                    )
                    nc.vector.scalar_tensor_tensor(
                        msk[:, :F], v[:, :F], -1.0, msk[:, :F],
                        Alu.is_gt, Alu.logical_and,
                    )
                    nc.vector.scalar_tensor_tensor(
                        msk[:, :F], v[:, :F], float(H), msk[:, :F],
                        Alu.is_lt, Alu.logical_and,
                    )

                    # floor of clamped coords (== trunc for the valid range):
                    # uc = clamp(u,0,hi); r = roundcast(uc); r -= (r > uc)
                    ui = wk_pool.tile([128, TILE], f32, tag="ui")
                    vi = wk_pool.tile([128, TILE], f32, tag="vi")
                    iu = wk_pool.tile([128, TILE], mybir.dt.int32, tag="iu")
                    iv = wk_pool.tile([128, TILE], mybir.dt.int32, tag="iv")
                    rf = wk_pool.tile([128, TILE], f32, tag="rf")
                    rg = wk_pool.tile([128, TILE], f32, tag="rg")

                    def floor_clamped(dst, src, hi_val, itile, rtile):
                        nc.vector.tensor_scalar(
                            dst[:, :F], src[:, :F], 0.0, hi_val,
                            Alu.max, Alu.min,
                        )
                        nc.scalar.copy(out=itile[:, :F], in_=dst[:, :F])
                        nc.scalar.copy(out=rtile[:, :F], in_=itile[:, :F])
                        nc.vector.tensor_tensor(
                            out=itile[:, :F].bitcast(f32),
                            in0=rtile[:, :F],
                            in1=dst[:, :F],
                            op=Alu.is_gt,
                        )
                        nc.vector.tensor_sub(
                            dst[:, :F], rtile[:, :F], itile[:, :F].bitcast(f32)
                        )

                    floor_clamped(ui, u, float(W), iu, rf)
                    floor_clamped(vi, v, float(H), iv, rg)

                    # pix = valid ? vi*W + ui : H*W   (+H*W fused into cast)
                    pixf = wk_pool.tile([128, TILE], f32, tag="pixf")
                    nc.vector.scalar_tensor_tensor(
                        pixf[:, :F], vi[:, :F], float(W), ui[:, :F],
                        Alu.mult, Alu.add,
                    )
                    nc.vector.tensor_scalar(
                        pixf[:, :F], pixf[:, :F], float(H * W), None,
                        Alu.subtract,
                    )
                    nc.vector.tensor_mul(pixf[:, :F], pixf[:, :F], msk[:, :F])
                    pixi = wk_pool.tile([128, TILE], mybir.dt.int32, tag="pixi")
                    nc.scalar.activation(
                        out=pixi[:, :F],
                        in_=pixf[:, :F],
                        func=mybir.ActivationFunctionType.Copy,
                        bias=float(H * W),
                    )

                    nc.sync.dma_start(
                        out=pix_out[img, :, lo:hi], in_=pixi[:, :F]
                    )
                    nc.sync.dma_start(
                        out=dep_out[img, :, lo:hi], in_=vcz[:, :F]
                    )
    return nc


def _get_nc():
    if "nc" not in _NC_CACHE:
        _NC_CACHE["nc"] = _build_nc()
    return _NC_CACHE["nc"]


def kernel(vertices, rotation, translation, camera_intrinsics):
    global LAST_RESULTS
    from concourse.bass_utils import run_bass_kernel_spmd

    vertices = np.ascontiguousarray(vertices, dtype=np.float32)
    rotation = np.asarray(rotation, dtype=np.float32)
    translation = np.asarray(translation, dtype=np.float32)
    camera_intrinsics = np.asarray(camera_intrinsics, dtype=np.float32)

    in_maps = []
    for core in range(N_CORES):
        vimgs = []
        cimgs = []
        for j in range(IMGS_PER_CORE):
            b = core * IMGS_PER_CORE + j
            vp = np.full((NPAD, 3), np.nan, dtype=np.float32)
            vp[:N] = vertices[b]
            # device layout: partition p holds points [p*COLS, (p+1)*COLS)
            vimgs.append(vp.reshape(128, COLS, 3))
            R = rotation[b]
            K = camera_intrinsics[b]
            c = np.zeros(16, dtype=np.float32)
            c[0:9] = R.reshape(9)
            c[9:12] = translation[b]
            c[12], c[13] = K[0, 0], K[1, 1]
            c[14], c[15] = K[0, 2], K[1, 2]
            cimgs.append(np.broadcast_to(c, (128, 16)).copy())
        vs = np.stack(vimgs)  # [IMGS, 128, COLS, 3]
        in_maps.append(
            {
                "vx": np.ascontiguousarray(vs[..., 0]),
                "vy": np.ascontiguousarray(vs[..., 1]),
                "vz": np.ascontiguousarray(vs[..., 2]),
                "consts": np.stack(cimgs),
            }
        )

    nc = _get_nc()
    import time as _time

    _t0 = _time.time()
    res = run_bass_kernel_spmd(nc, in_maps, core_ids=list(range(N_CORES)))
    globals()["LAST_EXEC_S"] = _time.time() - _t0
    LAST_RESULTS = res

    out = np.zeros((B, 1, H, W), dtype=np.float32)
    flat = out.reshape(B, H * W)
    for core in range(N_CORES):
        r = res.results[core]
        for j in range(IMGS_PER_CORE):
            b = core * IMGS_PER_CORE + j
            pixv = r["pix"][j].reshape(128 * COLS)[:N]
            depv = r["dep"][j].reshape(128 * COLS)[:N]
            m = (pixv >= 0) & (pixv < H * W)
            # sequential fancy assignment: later duplicates overwrite earlier
            flat[b][pixv[m]] = depv[m]
    return out


# revision 13
# speedup vs baseline: 1.1699x; 1.1204x over previous
"""Trainium2 kernel for nn_DifferentiableRenderer: batch-parallel point
projection + z-buffer scatter (last-write-wins).

Sharding: pure data parallel — B=16 images across 8 NeuronCores (2 each).
Device computes the memory-bound projection (world->camera transform,
perspective divide, pixel index + validity) for all 500K points per image;
per-pixel winner resolution is applied on the gathered per-point
(pixel, depth) arrays.
"""

import numpy as np

# ---------------------------------------------------------------------------
# TileContext compatibility patch: the walrus build in this environment
# rejects instructions carrying more than one sync-wait ("Too many sync wait
# commands") and Drain instructions with waits. Replace the Tile kernel-tail
# drain+barrier, and split any multi-wait instruction that slips through.
# ---------------------------------------------------------------------------


def _install_tile_patch():
    from concourse.tile import TileContext
    from concourse.vector_clock import ScopedClock, VectorClock

    if getattr(TileContext, "_render_patch", False):
        return

    def _patched_drain_and_barrier(self, tick_clock, wait_clock):
        nc = self.nc
        vec = list(tick_clock.global_clock)
        for proc, tick in enumerate(vec):
            if tick > 0:
                v = [0] * len(vec)
                v[proc] = tick
                nop = nc.sync.nop(nofuse=True)
                wait_clock.add_sem_waits(
                    nop.ins, ScopedClock({None: VectorClock(v)})
                )
        nc.all_engine_barrier(sem_only=True)
        popped = nc._tile_sem_poison_stack.pop()
        assert popped is self._sem_poison
        sems = list(self.sems.allocated().values())
        sem_nums = sorted(s.num if hasattr(s, "num") else int(s) for s in sems)
        if sem_nums:
            from concourse.bass import compact_to_ranges

            for r in compact_to_ranges(sem_nums):
                nc.gpsimd.sem_clear(r)
            nc._state.prepend_free_semaphores(sem_nums)
            for poison_set in nc._tile_sem_poison_stack:
                poison_set.update(sem_nums)
        nc.all_engine_barrier(sem_only=True)

    _orig_lower = TileContext._lower_ordered_insts

    def _split_multi_waits(self, ordered):
        import concourse.mybir as mybir

        for bb_name, insts in ordered.items():
            i = 0
            while i < len(insts):
                ins = insts[i]
                si = ins.sync_info
                if si is not None and len(si.on_wait) > 1:
                    waits = list(si.on_wait)
                    carriers = []
                    for w in waits[:-1]:
                        nop = mybir.InstNoOp(
                            name=f"I-{self.nc.next_id()}-ws", ins=[], outs=[]
                        )
                        nop.engine = ins.engine
                        nop.sync_info = mybir.SyncInfo(on_wait=[w], on_update=[])
                        carriers.append(nop)
                    ins.sync_info = mybir.SyncInfo(
                        on_wait=[waits[-1]], on_update=list(si.on_update)
                    )
                    insts[i:i] = carriers
                    i += len(carriers)
                i += 1
        return ordered

    def _patched_lower(self, ordered):
        return _orig_lower(self, _split_multi_waits(self, ordered))

    TileContext._drain_and_barrier = _patched_drain_and_barrier
    TileContext._lower_ordered_insts = _patched_lower
    TileContext._render_patch = True


# ---------------------------------------------------------------------------
# Problem constants (hardcoded per the task contract)
# ---------------------------------------------------------------------------
B, N = 16, 500000
H, W = 224, 224
N_CORES = 8
IMGS_PER_CORE = B // N_CORES  # 2
NPAD = ((N + 127) // 128) * 128  # 500096, multiple of 128
COLS = NPAD // 128  # 3907 columns per partition per image
TILE = 512
NTILES = (COLS + TILE - 1) // TILE

_NC_CACHE = {}
LAST_RESULTS = None


def _build_nc():
    """Per-core Bass program: for each of 2 images, project NPAD points ->
    per-point pixel index (int32, OOB=H*W) and depth (f32)."""
    import concourse.bass as bass
    import concourse.mybir as mybir
    from concourse.tile import TileContext

    _install_tile_patch()

    nc = bass.Bass()
    f32 = mybir.dt.float32
    Alu = mybir.AluOpType
    vx_in = nc.dram_tensor(
        "vx", [IMGS_PER_CORE, 128, COLS], f32, kind="ExternalInput"
    )
    vy_in = nc.dram_tensor(
        "vy", [IMGS_PER_CORE, 128, COLS], f32, kind="ExternalInput"
    )
    vz_in = nc.dram_tensor(
        "vz", [IMGS_PER_CORE, 128, COLS], f32, kind="ExternalInput"
    )
    # 16 scalars per image, pre-replicated across 128 partitions on host
    consts = nc.dram_tensor(
        "consts", [IMGS_PER_CORE, 128, 16], f32, kind="ExternalInput"
    )
    pix_out = nc.dram_tensor(
        "pix", [IMGS_PER_CORE, 128, COLS], mybir.dt.int32, kind="ExternalOutput"
    )
    dep_out = nc.dram_tensor(
        "dep", [IMGS_PER_CORE, 128, COLS], f32, kind="ExternalOutput"
    )

    with TileContext(nc) as tc:
        with (
            tc.tile_pool(name="io", bufs=3) as io_pool,
            tc.tile_pool(name="wk", bufs=2) as wk_pool,
            tc.tile_pool(name="cs", bufs=1) as cs_pool,
        ):
            cvec = []
            for img in range(IMGS_PER_CORE):
                cbc = cs_pool.tile([128, 16], f32, tag=f"cbc{img}")
                nc.sync.dma_start(out=cbc[:], in_=consts[img])
                cvec.append(cbc)

            for img in range(IMGS_PER_CORE):
                cb = cvec[img]
                r00, r01, r02 = cb[:, 0:1], cb[:, 1:2], cb[:, 2:3]
                r10, r11, r12 = cb[:, 3:4], cb[:, 4:5], cb[:, 5:6]
                r20, r21, r22 = cb[:, 6:7], cb[:, 7:8], cb[:, 8:9]
                tx, ty, tz = cb[:, 9:10], cb[:, 10:11], cb[:, 11:12]
                fx, fy = cb[:, 12:13], cb[:, 13:14]
                cx, cy = cb[:, 14:15], cb[:, 15:16]

                for t in range(NTILES):
                    lo = t * TILE
                    hi = min(COLS, lo + TILE)
                    F = hi - lo
                    x = io_pool.tile([128, TILE], f32, tag="x")
                    y = io_pool.tile([128, TILE], f32, tag="y")
                    z = io_pool.tile([128, TILE], f32, tag="z")
                    nc.sync.dma_start(out=x[:, :F], in_=vx_in[img, :, lo:hi])
                    nc.sync.dma_start(out=y[:, :F], in_=vy_in[img, :, lo:hi])
                    nc.sync.dma_start(out=z[:, :F], in_=vz_in[img, :, lo:hi])

                    xs, ys, zs = x[:, :F], y[:, :F], z[:, :F]

                    vcx = wk_pool.tile([128, TILE], f32, tag="vcx")
                    vcy = wk_pool.tile([128, TILE], f32, tag="vcy")
                    vcz = wk_pool.tile([128, TILE], f32, tag="vcz")
                    tmp = wk_pool.tile([128, TILE], f32, tag="tmp")
                    Act = mybir.ActivationFunctionType

                    def mad3(out, ra, rb, rc, tt):
                        # out = ((x*ra + y*rb) + z*rc) + tt -- same order as
                        # the reference einsum + translation add. First mul
                        # and final add run on ACT to offload DVE.
                        nc.vector.tensor_scalar_mul(out[:, :F], xs, ra)
                        nc.vector.scalar_tensor_tensor(
                            out[:, :F], ys, rb, out[:, :F], Alu.mult, Alu.add
                        )
                        nc.vector.scalar_tensor_tensor(
                            out[:, :F], zs, rc, out[:, :F], Alu.mult, Alu.add
                        )
                        nc.vector.tensor_scalar(
                            out[:, :F], out[:, :F], tt, None, Alu.add
                        )

                    mad3(vcx, r00, r01, r02, tx)
                    mad3(vcy, r10, r11, r12, ty)
                    mad3(vcz, r20, r21, r22, tz)

                    # zr = 1/(vcz + 1e-8), one Newton refinement
                    zb = wk_pool.tile([128, TILE], f32, tag="zb")
                    nc.vector.tensor_scalar(
                        zb[:, :F], vcz[:, :F], 1e-8, None, Alu.add
                    )
                    zr = wk_pool.tile([128, TILE], f32, tag="zr")
                    nc.vector.reciprocal(out=zr[:, :F], in_=zb[:, :F])
                    nc.vector.tensor_mul(tmp[:, :F], zb[:, :F], zr[:, :F])
                    nc.vector.tensor_scalar(
                        tmp[:, :F], tmp[:, :F], 2.0, -1.0,
                        Alu.subtract, Alu.mult,
                    )
                    nc.vector.tensor_mul(zr[:, :F], zr[:, :F], tmp[:, :F])

                    u = wk_pool.tile([128, TILE], f32, tag="u")
                    v = wk_pool.tile([128, TILE], f32, tag="v")
                    nc.vector.tensor_mul(u[:, :F], vcx[:, :F], zr[:, :F])
                    nc.vector.tensor_scalar(
                        u[:, :F], u[:, :F], fx, cx, Alu.mult, Alu.add
                    )
                    nc.vector.tensor_mul(v[:, :F], vcy[:, :F], zr[:, :F])
                    nc.vector.tensor_scalar(
                        v[:, :F], v[:, :F], fy, cy, Alu.mult, Alu.add
                    )

                    # border-encoded trunc: clamp to [-1, hi], floor, then
                    # pix226 = (vi+1)*226 + (ui+1); rows/cols 0 and 225 mark
                    # invalid (decoded on the host). floor(x) = roundcast(x)
                    # minus (rounded > x); exact for the clamp range.
                    ui = wk_pool.tile([128, TILE], f32, tag="ui")
                    vi = wk_pool.tile([128, TILE], f32, tag="vi")
                    iu = wk_pool.tile([128, TILE], mybir.dt.int32, tag="iu")
                    iv = wk_pool.tile([128, TILE], mybir.dt.int32, tag="iv")
                    rf = wk_pool.tile([128, TILE], f32, tag="rf")
                    rg = wk_pool.tile([128, TILE], f32, tag="rg")

                    def border_code(dst, src, hi_val, itile, rtile):
                        # dst = floor(clamp(src, 0, hi)) + (src > -1):
                        # 0 when src <= -1 (invalid-low), hi+1 when src >= hi
                        # (invalid-high), else trunc(src)+1 -- matching the
                        # reference's trunc-toward-zero validity exactly.
                        nc.vector.tensor_scalar(
                            dst[:, :F], src[:, :F], 0.0, hi_val,
                            Alu.max, Alu.min,
                        )
                        nc.scalar.copy(out=itile[:, :F], in_=dst[:, :F])
                        nc.scalar.copy(out=rtile[:, :F], in_=itile[:, :F])
                        nc.vector.tensor_tensor(
                            out=itile[:, :F].bitcast(f32),
                            in0=rtile[:, :F],
                            in1=dst[:, :F],
                            op=Alu.is_gt,
                        )
                        nc.vector.tensor_sub(
                            dst[:, :F], rtile[:, :F], itile[:, :F].bitcast(f32)
                        )
                        nc.vector.scalar_tensor_tensor(
                            dst[:, :F], src[:, :F], -1.0, dst[:, :F],
                            Alu.is_gt, Alu.add,
                        )

                    border_code(ui, u, float(W), iu, rf)
                    border_code(vi, v, float(H), iv, rg)

                    pixf = wk_pool.tile([128, TILE], f32, tag="pixf")
                    nc.vector.scalar_tensor_tensor(
                        pixf[:, :F], vi[:, :F], 226.0, ui[:, :F],
                        Alu.mult, Alu.add,
                    )
                    pixi = wk_pool.tile([128, TILE], mybir.dt.int32, tag="pixi")
                    nc.scalar.copy(out=pixi[:, :F], in_=pixf[:, :F])

                    nc.sync.dma_start(
                        out=pix_out[img, :, lo:hi], in_=pixi[:, :F]
                    )
                    nc.sync.dma_start(
                        out=dep_out[img, :, lo:hi], in_=vcz[:, :F]
                    )
    return nc


def _get_nc():
    if "nc" not in _NC_CACHE:
        _NC_CACHE["nc"] = _build_nc()
    return _NC_CACHE["nc"]


def kernel(vertices, rotation, translation, camera_intrinsics):
    global LAST_RESULTS
    from concourse.bass_utils import run_bass_kernel_spmd

    vertices = np.ascontiguousarray(vertices, dtype=np.float32)
    rotation = np.asarray(rotation, dtype=np.float32)
    translation = np.asarray(translation, dtype=np.float32)
    camera_intrinsics = np.asarray(camera_intrinsics, dtype=np.float32)

    in_maps = []
    for core in range(N_CORES):
        vimgs = []
        cimgs = []
        for j in range(IMGS_PER_CORE):
            b = core * IMGS_PER_CORE + j
            vp = np.full((NPAD, 3), np.nan, dtype=np.float32)
            vp[:N] = vertices[b]
            # device layout: partition p holds points [p*COLS, (p+1)*COLS)
            vimgs.append(vp.reshape(128, COLS, 3))
            R = rotation[b]
            K = camera_intrinsics[b]
            c = np.zeros(16, dtype=np.float32)
            c[0:9] = R.reshape(9)
            c[9:12] = translation[b]
            c[12], c[13] = K[0, 0], K[1, 1]
            c[14], c[15] = K[0, 2], K[1, 2]
            cimgs.append(np.broadcast_to(c, (128, 16)).copy())
        vs = np.stack(vimgs)  # [IMGS, 128, COLS, 3]
        in_maps.append(
            {
                "vx": np.ascontiguousarray(vs[..., 0]),
                "vy": np.ascontiguousarray(vs[..., 1]),
                "vz": np.ascontiguousarray(vs[..., 2]),
                "consts": np.stack(cimgs),
            }
        )

    nc = _get_nc()
    import time as _time

    _t0 = _time.time()
    res = run_bass_kernel_spmd(nc, in_maps, core_ids=list(range(N_CORES)))
    globals()["LAST_EXEC_S"] = _time.time() - _t0
    LAST_RESULTS = res

    out = np.zeros((B, 1, H, W), dtype=np.float32)
    flat = out.reshape(B, H * W)
    for core in range(N_CORES):
        r = res.results[core]
        for j in range(IMGS_PER_CORE):
            b = core * IMGS_PER_CORE + j
            p226 = r["pix"][j].reshape(128 * COLS)[:N]
            depv = r["dep"][j].reshape(128 * COLS)[:N]
            # decode border-encoded index: p226 = (vi+1)*226 + (ui+1) with
            # vi/ui clamped to [-1, 224]; rows/cols 0 and 225 are invalid
            row = p226 // 226 - 1
            col = p226 % 226 - 1
            m = (row >= 0) & (row < H) & (col >= 0) & (col < W)
            pixv = row * W + col
            # sequential fancy assignment: later duplicates overwrite earlier
            flat[b][pixv[m]] = depv[m]
    return out


# revision 15
# speedup vs baseline: 1.2316x; 1.0527x over previous
"""Trainium2 kernel for nn_DifferentiableRenderer: batch-parallel point
projection + z-buffer scatter (last-write-wins).

Sharding: pure data parallel — B=16 images across 8 NeuronCores (2 each).
Device computes the memory-bound projection (world->camera transform,
perspective divide, pixel index + validity) for all 500K points per image;
per-pixel winner resolution is applied on the gathered per-point
(pixel, depth) arrays.
"""

import numpy as np

# ---------------------------------------------------------------------------
# TileContext compatibility patch: the walrus build in this environment
# rejects instructions carrying more than one sync-wait ("Too many sync wait
# commands") and Drain instructions with waits. Replace the Tile kernel-tail
# drain+barrier, and split any multi-wait instruction that slips through.
# ---------------------------------------------------------------------------


def _install_tile_patch():
    from concourse.tile import TileContext
    from concourse.vector_clock import ScopedClock, VectorClock

    if getattr(TileContext, "_render_patch", False):
        return

    def _patched_drain_and_barrier(self, tick_clock, wait_clock):
        nc = self.nc
        vec = list(tick_clock.global_clock)
        for proc, tick in enumerate(vec):
            if tick > 0:
                v = [0] * len(vec)
                v[proc] = tick
                nop = nc.sync.nop(nofuse=True)
                wait_clock.add_sem_waits(
                    nop.ins, ScopedClock({None: VectorClock(v)})
                )
        nc.all_engine_barrier(sem_only=True)
        popped = nc._tile_sem_poison_stack.pop()
        assert popped is self._sem_poison
        sems = list(self.sems.allocated().values())
        sem_nums = sorted(s.num if hasattr(s, "num") else int(s) for s in sems)
        if sem_nums:
            from concourse.bass import compact_to_ranges

            for r in compact_to_ranges(sem_nums):
                nc.gpsimd.sem_clear(r)
            nc._state.prepend_free_semaphores(sem_nums)
            for poison_set in nc._tile_sem_poison_stack:
                poison_set.update(sem_nums)
        nc.all_engine_barrier(sem_only=True)

    _orig_lower = TileContext._lower_ordered_insts

    def _split_multi_waits(self, ordered):
        import concourse.mybir as mybir

        for bb_name, insts in ordered.items():
            i = 0
            while i < len(insts):
                ins = insts[i]
                si = ins.sync_info
                if si is not None and len(si.on_wait) > 1:
                    waits = list(si.on_wait)
                    carriers = []
                    for w in waits[:-1]:
                        nop = mybir.InstNoOp(
                            name=f"I-{self.nc.next_id()}-ws", ins=[], outs=[]
                        )
                        nop.engine = ins.engine
                        nop.sync_info = mybir.SyncInfo(on_wait=[w], on_update=[])
                        carriers.append(nop)
                    ins.sync_info = mybir.SyncInfo(
                        on_wait=[waits[-1]], on_update=list(si.on_update)
                    )
                    insts[i:i] = carriers
                    i += len(carriers)
                i += 1
        return ordered

    def _patched_lower(self, ordered):
        return _orig_lower(self, _split_multi_waits(self, ordered))

    TileContext._drain_and_barrier = _patched_drain_and_barrier
    TileContext._lower_ordered_insts = _patched_lower
    TileContext._render_patch = True


# ---------------------------------------------------------------------------
# Problem constants (hardcoded per the task contract)
# ---------------------------------------------------------------------------
B, N = 16, 500000
H, W = 224, 224
N_CORES = 8
IMGS_PER_CORE = B // N_CORES  # 2
NPAD = ((N + 127) // 128) * 128  # 500096, multiple of 128
COLS = NPAD // 128  # 3907 columns per partition per image
TILE = 1024
NTILES = (COLS + TILE - 1) // TILE

_NC_CACHE = {}
LAST_RESULTS = None


def _build_nc():
    """Per-core Bass program: for each of 2 images, project NPAD points ->
    per-point pixel index (int32, OOB=H*W) and depth (f32)."""
    import concourse.bass as bass
    import concourse.mybir as mybir
    from concourse.tile import TileContext

    _install_tile_patch()

    nc = bass.Bass()
    f32 = mybir.dt.float32
    Alu = mybir.AluOpType
    vx_in = nc.dram_tensor(
        "vx", [IMGS_PER_CORE, 128, COLS], f32, kind="ExternalInput"
    )
    vy_in = nc.dram_tensor(
        "vy", [IMGS_PER_CORE, 128, COLS], f32, kind="ExternalInput"
    )
    vz_in = nc.dram_tensor(
        "vz", [IMGS_PER_CORE, 128, COLS], f32, kind="ExternalInput"
    )
    # 16 scalars per image, pre-replicated across 128 partitions on host
    consts = nc.dram_tensor(
        "consts", [IMGS_PER_CORE, 128, 16], f32, kind="ExternalInput"
    )
    pix_out = nc.dram_tensor(
        "pix", [IMGS_PER_CORE, 128, COLS], mybir.dt.int32, kind="ExternalOutput"
    )
    dep_out = nc.dram_tensor(
        "dep", [IMGS_PER_CORE, 128, COLS], f32, kind="ExternalOutput"
    )

    with TileContext(nc) as tc:
        with (
            tc.tile_pool(name="io", bufs=3) as io_pool,
            tc.tile_pool(name="wk", bufs=2) as wk_pool,
            tc.tile_pool(name="cs", bufs=1) as cs_pool,
        ):
            cvec = []
            for img in range(IMGS_PER_CORE):
                cbc = cs_pool.tile([128, 16], f32, tag=f"cbc{img}")
                nc.sync.dma_start(out=cbc[:], in_=consts[img])
                cvec.append(cbc)

            for img in range(IMGS_PER_CORE):
                cb = cvec[img]
                r00, r01, r02 = cb[:, 0:1], cb[:, 1:2], cb[:, 2:3]
                r10, r11, r12 = cb[:, 3:4], cb[:, 4:5], cb[:, 5:6]
                r20, r21, r22 = cb[:, 6:7], cb[:, 7:8], cb[:, 8:9]
                tx, ty, tz = cb[:, 9:10], cb[:, 10:11], cb[:, 11:12]
                fx, fy = cb[:, 12:13], cb[:, 13:14]
                cx, cy = cb[:, 14:15], cb[:, 15:16]

                for t in range(NTILES):
                    lo = t * TILE
                    hi = min(COLS, lo + TILE)
                    F = hi - lo
                    x = io_pool.tile([128, TILE], f32, tag="x")
                    y = io_pool.tile([128, TILE], f32, tag="y")
                    z = io_pool.tile([128, TILE], f32, tag="z")
                    nc.sync.dma_start(out=x[:, :F], in_=vx_in[img, :, lo:hi])
                    nc.sync.dma_start(out=y[:, :F], in_=vy_in[img, :, lo:hi])
                    nc.sync.dma_start(out=z[:, :F], in_=vz_in[img, :, lo:hi])

                    xs, ys, zs = x[:, :F], y[:, :F], z[:, :F]

                    vcx = wk_pool.tile([128, TILE], f32, tag="vcx")
                    vcy = wk_pool.tile([128, TILE], f32, tag="vcy")
                    vcz = wk_pool.tile([128, TILE], f32, tag="vcz")
                    tmp = wk_pool.tile([128, TILE], f32, tag="tmp")
                    Act = mybir.ActivationFunctionType

                    def mad3(out, ra, rb, rc, tt):
                        # out = ((x*ra + y*rb) + z*rc) + tt -- same order as
                        # the reference einsum + translation add. First mul
                        # and final add run on ACT to offload DVE.
                        nc.vector.tensor_scalar_mul(out[:, :F], xs, ra)
                        nc.vector.scalar_tensor_tensor(
                            out[:, :F], ys, rb, out[:, :F], Alu.mult, Alu.add
                        )
                        nc.vector.scalar_tensor_tensor(
                            out[:, :F], zs, rc, out[:, :F], Alu.mult, Alu.add
                        )
                        nc.vector.tensor_scalar(
                            out[:, :F], out[:, :F], tt, None, Alu.add
                        )

                    mad3(vcx, r00, r01, r02, tx)
                    mad3(vcy, r10, r11, r12, ty)
                    mad3(vcz, r20, r21, r22, tz)

                    # zr = 1/(vcz + 1e-8), one Newton refinement
                    zb = wk_pool.tile([128, TILE], f32, tag="zb")
                    nc.vector.tensor_scalar(
                        zb[:, :F], vcz[:, :F], 1e-8, None, Alu.add
                    )
                    zr = wk_pool.tile([128, TILE], f32, tag="zr")
                    nc.vector.reciprocal(out=zr[:, :F], in_=zb[:, :F])
                    nc.vector.tensor_mul(tmp[:, :F], zb[:, :F], zr[:, :F])
                    nc.vector.tensor_scalar(
                        tmp[:, :F], tmp[:, :F], 2.0, -1.0,
                        Alu.subtract, Alu.mult,
                    )
                    nc.vector.tensor_mul(zr[:, :F], zr[:, :F], tmp[:, :F])

                    u = wk_pool.tile([128, TILE], f32, tag="u")
                    v = wk_pool.tile([128, TILE], f32, tag="v")
                    nc.vector.tensor_mul(u[:, :F], vcx[:, :F], zr[:, :F])
                    nc.vector.tensor_scalar(
                        u[:, :F], u[:, :F], fx, cx, Alu.mult, Alu.add
                    )
                    nc.vector.tensor_mul(v[:, :F], vcy[:, :F], zr[:, :F])
                    nc.vector.tensor_scalar(
                        v[:, :F], v[:, :F], fy, cy, Alu.mult, Alu.add
                    )

                    # border-encoded trunc: clamp to [-1, hi], floor, then
                    # pix226 = (vi+1)*226 + (ui+1); rows/cols 0 and 225 mark
                    # invalid (decoded on the host). floor(x) = roundcast(x)
                    # minus (rounded > x); exact for the clamp range.
                    ui = wk_pool.tile([128, TILE], f32, tag="ui")
                    vi = wk_pool.tile([128, TILE], f32, tag="vi")
                    iu = wk_pool.tile([128, TILE], mybir.dt.int32, tag="iu")
                    iv = wk_pool.tile([128, TILE], mybir.dt.int32, tag="iv")
                    rf = wk_pool.tile([128, TILE], f32, tag="rf")
                    rg = wk_pool.tile([128, TILE], f32, tag="rg")

                    def border_code(dst, src, hi_val, itile, rtile):
                        # dst = floor(clamp(src, 0, hi)) + (src > -1):
                        # 0 when src <= -1 (invalid-low), hi+1 when src >= hi
                        # (invalid-high), else trunc(src)+1 -- matching the
                        # reference's trunc-toward-zero validity exactly.
                        nc.vector.tensor_scalar(
                            dst[:, :F], src[:, :F], 0.0, hi_val,
                            Alu.max, Alu.min,
                        )
                        nc.scalar.copy(out=itile[:, :F], in_=dst[:, :F])
                        nc.scalar.copy(out=rtile[:, :F], in_=itile[:, :F])
                        nc.vector.tensor_tensor(
                            out=itile[:, :F].bitcast(f32),
                            in0=rtile[:, :F],
                            in1=dst[:, :F],
                            op=Alu.is_gt,
                        )
                        nc.vector.tensor_sub(
                            dst[:, :F], rtile[:, :F], itile[:, :F].bitcast(f32)
                        )
                        nc.vector.scalar_tensor_tensor(
                            dst[:, :F], src[:, :F], -1.0, dst[:, :F],
                            Alu.is_gt, Alu.add,
                        )

                    border_code(ui, u, float(W), iu, rf)
                    border_code(vi, v, float(H), iv, rg)

                    pixf = wk_pool.tile([128, TILE], f32, tag="pixf")
                    nc.vector.scalar_tensor_tensor(
                        pixf[:, :F], vi[:, :F], 226.0, ui[:, :F],
                        Alu.mult, Alu.add,
                    )
                    pixi = wk_pool.tile([128, TILE], mybir.dt.int32, tag="pixi")
                    nc.scalar.copy(out=pixi[:, :F], in_=pixf[:, :F])

                    nc.sync.dma_start(
                        out=pix_out[img, :, lo:hi], in_=pixi[:, :F]
                    )
                    nc.sync.dma_start(
                        out=dep_out[img, :, lo:hi], in_=vcz[:, :F]
                    )
    return nc


def _get_nc():
    if "nc" not in _NC_CACHE:
        _NC_CACHE["nc"] = _build_nc()
    return _NC_CACHE["nc"]


def kernel(vertices, rotation, translation, camera_intrinsics):
    global LAST_RESULTS
    from concourse.bass_utils import run_bass_kernel_spmd

    vertices = np.ascontiguousarray(vertices, dtype=np.float32)
    rotation = np.asarray(rotation, dtype=np.float32)
    translation = np.asarray(translation, dtype=np.float32)
    camera_intrinsics = np.asarray(camera_intrinsics, dtype=np.float32)

    in_maps = []
    for core in range(N_CORES):
        vimgs = []
        cimgs = []
        for j in range(IMGS_PER_CORE):
            b = core * IMGS_PER_CORE + j
            vp = np.full((NPAD, 3), np.nan, dtype=np.float32)
            vp[:N] = vertices[b]
            # device layout: partition p holds points [p*COLS, (p+1)*COLS)
            vimgs.append(vp.reshape(128, COLS, 3))
            R = rotation[b]
            K = camera_intrinsics[b]
            c = np.zeros(16, dtype=np.float32)
            c[0:9] = R.reshape(9)
            c[9:12] = translation[b]
            c[12], c[13] = K[0, 0], K[1, 1]
            c[14], c[15] = K[0, 2], K[1, 2]
            cimgs.append(np.broadcast_to(c, (128, 16)).copy())
        vs = np.stack(vimgs)  # [IMGS, 128, COLS, 3]
        in_maps.append(
            {
                "vx": np.ascontiguousarray(vs[..., 0]),
                "vy": np.ascontiguousarray(vs[..., 1]),
                "vz": np.ascontiguousarray(vs[..., 2]),
                "consts": np.stack(cimgs),
            }
        )

    nc = _get_nc()
    import time as _time

    _t0 = _time.time()
    res = run_bass_kernel_spmd(nc, in_maps, core_ids=list(range(N_CORES)))
    globals()["LAST_EXEC_S"] = _time.time() - _t0
    LAST_RESULTS = res

    out = np.zeros((B, 1, H, W), dtype=np.float32)
    flat = out.reshape(B, H * W)
    for core in range(N_CORES):
        r = res.results[core]
        for j in range(IMGS_PER_CORE):
            b = core * IMGS_PER_CORE + j
            p226 = r["pix"][j].reshape(128 * COLS)[:N]
            depv = r["dep"][j].reshape(128 * COLS)[:N]
            # decode border-encoded index: p226 = (vi+1)*226 + (ui+1) with
            # vi/ui clamped to [-1, 224]; rows/cols 0 and 225 are invalid
            row = p226 // 226 - 1
            col = p226 % 226 - 1
            m = (row >= 0) & (row < H) & (col >= 0) & (col < W)
            pixv = row * W + col
            # sequential fancy assignment: later duplicates overwrite earlier
            flat[b][pixv[m]] = depv[m]
    return out
